# revision 33
# baseline (speedup 1.0000x reference)
"""LSTM cell kernel for Trainium2, SPMD over 8 NeuronCores — fp8 DoubleRow.

Problem: nn_LstmCell — B=8192, D_IN=D_H=2048.
    g = x @ Wx.T + bx + h @ Wh.T + bh          # [B, 3H]
    gi, gm, go = split(g, 3)
    c_new = sigmoid(gm)*c + sigmoid(gi)*tanh(gm)
    h_new = sigmoid(go)*tanh(c_new)

Strategy:
  - Data-parallel over batch: each core owns 1024 rows of x/h/c.
  - Single fused GEMM A = [x ‖ h] (K=4096) against W = [Wx ‖ Wh] ([6144, 4096]),
    computed transposed (gates on PSUM partitions, batch on free dim).
  - fp8(e4m3) matmuls in MatmulPerfMode.DoubleRow: one instruction contracts a
    PAIR of 128-deep k-planes at 0.5 cycles per output row — 4x the bf16 rate.
  - Mixed precision to stay inside the error budget: the memory gate (gm,
    whose pre-activation feeds both sigmoid and tanh) gets a first-order
    quantization-error correction; gi/go run raw fp8.
      A = A8 + A8l, W = W8 + W8l (fp8 residual decomposition, same scale so
      everything accumulates in one PSUM):
        gm  ≈ W8ᵀA8 + W8lᵀA8 (all 16 k-pairs) + W8ᵀA8l (first XCORR_PAIRS)
        gi/go ≈ W8ᵀA8                        (1 DR matmul per pair)
    Measured on HW (bit-exact vs the numpy pipeline sim): h_new rel err
    1.890e-2 at XCORR_PAIRS=8 (budget 2e-2).
  - Schedule: m -> i -> o per d-tile with eager per-gate PSUM drains; m-gate
    PSUM banks double-buffered by d-parity (8 banks exactly); d0 consumes A8
    chunks as they stream, d0/d1 defer the x-corr until the A8l planes land;
    weight/c/out DMAs sequenced so the serialized DMA engines never starve
    the PE after startup.
  - Per-gate bias and the 2^-18 dequant scale fold into the ScalarE
    activation (func(in*scale + bias) reads PSUM directly).
  - c streams in bf16; outputs are written in bf16 (upcast on host).

Host-side: layout transforms + fp8/bf16 casts (not counted in HW exec time).
"""

import os

import numpy as np
import ml_dtypes

N_CORES = 8
B = 8192
DH = 2048            # latent dim (= D_IN = D_H)
H3 = 3 * DH          # 6144 gate rows
K = 2 * DH           # 4096 contraction dim
BLOC = B // N_CORES  # 1024 batch rows per core
P = 128
KT = K // P          # 32 k-planes
NPAIR = KT // 2      # 16 DoubleRow k-plane pairs
MT = H3 // P         # 48 gate-row tiles
DTL = DH // P        # 16 d-tiles per gate
NF = 512             # matmul free dim (one PSUM bank of fp32)
NH = BLOC // NF      # 2 batch halves

SA = 32.0            # fp8 scale for A (|A|max ~5.1 -> 163 < 240)
SW = 8192.0          # fp8 scale for W (|W|max 0.0221 -> 181 < 240)
SC = 1.0 / (SA * SW)  # dequant scale folded into ACT

# k-pairs (of 16) that get the gm x-residual correction. 16 -> h_rel 1.50e-2,
# 8 -> 1.89e-2 (exact-pipeline sim, reproduces HW bit-for-bit), budget 2e-2.
XCORR_PAIRS = 8

_BF16 = ml_dtypes.bfloat16
_E4M3 = ml_dtypes.float8_e4m3

_CACHE = {}
LAST_RESULT = None  # BassKernelResults from the most recent run (for test.py)


def _split_multiwaits(nc):
    """This container's walrus build rejects >1 sync-wait on an engine
    instruction ("Too many sync wait commands"). Split extra waits into
    standalone EventSemaphore instructions on the same engine immediately
    before the instruction (same stall semantics: engines are in-order)."""
    import concourse.mybir as mybir

    f = nc.m.functions[0]
    for blk in f.blocks:
        new_insts = []
        for inst in blk.instructions:
            si = getattr(inst, "sync_info", None)
            ow = list(si.on_wait) if (si is not None and si.on_wait) else []
            if len(ow) > 1:
                for w in ow[:-1]:
                    new_insts.append(
                        mybir.InstEventSemaphore(
                            name=nc.get_next_instruction_name(),
                            engine=inst.engine,
                            ins=[],
                            outs=[],
                            sync_info=mybir.SyncInfo(on_wait=[w], on_update=[]),
                        )
                    )
                inst.sync_info = mybir.SyncInfo(
                    on_wait=[ow[-1]], on_update=list(si.on_update)
                )
            new_insts.append(inst)
        blk.instructions[:] = new_insts


def _build_bass():
    import concourse.bass as bass
    import concourse.mybir as mybir
    import concourse.tile as tile

    f32 = mybir.dt.float32
    bf16 = mybir.dt.bfloat16
    fp8 = mybir.dt.float8e4
    AF = mybir.ActivationFunctionType
    DR = mybir.MatmulPerfMode.DoubleRow

    nc = bass.Bass("TRN2", name="lstm_cell_fp8")

    # WP[d]: per-partition-contiguous pack of the 4 weight strips this d-tile
    # needs: [w8_i, w8_m, w8_o, w8l_m]. WP[d, p, s, kt, f] = strip_s[kt*128+p,
    # d-tile row f].
    WP = nc.dram_tensor("WP", [DTL, P, 4, KT, P], fp8, kind="ExternalInput")
    # AIL[p, kt, 0, n] = A8, AIL[p, kt, 1, n] = A8 residual.
    AIL = nc.dram_tensor("AIL", [P, KT, 2, BLOC], fp8, kind="ExternalInput")
    CTB = nc.dram_tensor("CTB", [DH, BLOC], bf16, kind="ExternalInput")
    BIAS = nc.dram_tensor("BIAS", [P, MT], f32, kind="ExternalInput")
    HT = nc.dram_tensor("HT", [DH, BLOC], bf16, kind="ExternalOutput")
    CNT = nc.dram_tensor("CNT", [DH, BLOC], bf16, kind="ExternalOutput")

    NCH = 8  # A-chunk DMAs per plane (2 k-pairs each)

    with tile.TileContext(nc) as tc:
        with (
            tc.tile_pool(name="const", bufs=1) as const_pool,
            tc.tile_pool(name="wpool", bufs=2) as wpool,
            tc.tile_pool(name="cpool", bufs=2) as cpool,
            tc.tile_pool(name="epool", bufs=3) as epool,
            tc.tile_pool(name="opool", bufs=2) as opool,
            tc.tile_pool(name="psum", bufs=1, space="PSUM") as psum_pool,
        ):
            # Activations resident in SBUF. A8 planes stream first (all
            # matmuls need them); A8l residual planes later (only the gm
            # x-correction reads them, and d0/d1 defer that).
            a_sb = const_pool.tile([P, KT, 2, BLOC], fp8, name="a_sb")

            # d0 weights as 4 separate strips (i first) so the first
            # matmuls start ~1.5us in instead of behind a 5.8us pack.
            strips0 = {}
            for si, sname in [(0, "i"), (1, "m"), (3, "lm"), (2, "o")]:
                st = const_pool.tile([P, KT, P], fp8, name=f"w0_{sname}")
                nc.sync.dma_start(st[:], WP[0][:, si])
                strips0[sname] = st
            bias_sb = const_pool.tile([P, MT], f32, name="bias_sb")
            nc.sync.dma_start(bias_sb[:], BIAS[:])

            w_tiles = {}
            c_tiles = {}

            def issue_w(d, strips=None):
                # strips: subset of slots to DMA on the in-order Pool queue
                # (sequencing them between A-chunk streams controls exactly
                # when they reach the serialized DMA engines); None = one
                # pack DMA on the SP queue.
                wp = w_tiles.get(d)
                if wp is None:
                    wp = wpool.tile([P, 4, KT, P], fp8, name="wp", tag="wp")
                    w_tiles[d] = wp
                if strips is None:
                    nc.sync.dma_start(wp[:], WP[d])
                else:
                    for s in strips:
                        nc.gpsimd.dma_start(wp[:, s], WP[d][:, s])

            def issue_c(d, pool_queue=False):
                c_t = cpool.tile([P, BLOC], bf16, name="c_t", tag="c_t")
                eng = nc.gpsimd if pool_queue else nc.sync
                eng.dma_start(c_t[:], CTB[d * P : (d + 1) * P, :])
                c_tiles[d] = c_t

            # Pool queue is in-order; sequence the startup byte stream so PE
            # work becomes available as early as possible: A8 chunks with d1's
            # m/lm strips early (d1-m matmuls interleave into the d0 sweep),
            # then the rest of d1, the A8l residual planes, d2, and the c's.
            def a_chunk(u, plane):
                nc.gpsimd.dma_start(
                    a_sb[:, 4 * u : 4 * u + 4, plane, :],
                    AIL[:, 4 * u : 4 * u + 4, plane, :],
                )

            for u in range(NCH):
                a_chunk(u, 0)
            issue_w(1, strips=(1, 3, 0, 2))
            for u in range((2 * XCORR_PAIRS + 3) // 4):  # chunks x-corr reads
                a_chunk(u, 1)
            issue_w(2, strips=(1, 3, 0, 2))
            issue_c(0, pool_queue=True)
            issue_c(1, pool_queue=True)

            def w_slice(d, s, t):
                if d == 0:
                    st = strips0[{0: "i", 1: "m", 2: "o", 3: "lm"}[s]]
                    return st[:, 2 * t : 2 * t + 2, :]
                return w_tiles[d][:, s, 2 * t : 2 * t + 2, :]

            def a_pair(t, plane, nh):
                return a_sb[:, 2 * t : 2 * t + 2, plane,
                            nh * NF : (nh + 1) * NF]

            def mm(psum, lhsT, rhs, start, stop):
                nc.tensor.matmul(psum, lhsT, rhs, start=start, stop=stop,
                                 perf_mode=DR)

            psums = {}

            def alloc_psums(d):
                # i/o banks single-buffered (recycled after the eager
                # s_i/s_o ACT read); m banks double-buffered by d-parity so
                # consecutive d-tiles' m accumulations never serialize.
                for g in "io":
                    for nh in range(NH):
                        psums[(d, g, nh)] = psum_pool.tile(
                            [P, NF], f32, name=f"ps_{g}{nh}", tag=f"ps_{g}{nh}"
                        )
                for nh in range(NH):
                    psums[(d, "m", nh)] = psum_pool.tile(
                        [P, NF], f32, name=f"ps_m{nh}", tag=f"ps_m{nh}_{d % 2}"
                    )

            def mm_gate(d, g, t, stop=False, nhs=(0, 1)):
                # plain gate: one DR per (t, nh) on the A8 plane
                gi = {"i": 0, "m": 1, "o": 2}[g]
                for nh in nhs:
                    mm(psums[(d, g, nh)][:], w_slice(d, gi, t),
                       a_pair(t, 0, nh), t == 0, stop and t == NPAIR - 1)

            def mm_m_main(d, t, start=None):
                for nh in range(NH):
                    mm(psums[(d, "m", nh)][:], w_slice(d, 1, t),
                       a_pair(t, 0, nh), t == 0 if start is None else start,
                       False)

            def mm_m_xcorr(d, t, stop=False):
                for nh in range(NH):
                    mm(psums[(d, "m", nh)][:], w_slice(d, 1, t),
                       a_pair(t, 1, nh), False, stop)

            def mm_m_wcorr(d, t, stop=False):
                for nh in range(NH):
                    mm(psums[(d, "m", nh)][:], w_slice(d, 3, t),
                       a_pair(t, 0, nh), False, stop and t == NPAIR - 1)

            def act_gate(d, g, func, dst, nh):
                col = {"i": d, "m": DTL + d, "o": 2 * DTL + d}[g]
                nc.scalar.activation(dst[:], psums[(d, g, nh)][:], func,
                                     bias=bias_sb[:, col : col + 1], scale=SC)

            sA = {}  # (d, name, nh) -> bf16 tile for s_i/t_m/s_m/s_o

            def act_io(d, g, nhs=(0, 1)):
                # eager phase-A drain: frees the i/o bank for d+1
                for nh in nhs:
                    tl = epool.tile([P, NF], bf16, name=f"s_{g}", tag=f"s_{g}{nh}")
                    act_gate(d, g, AF.Sigmoid, tl, nh)
                    sA[(d, f"s_{g}", nh)] = tl

            def act_m(d):
                for nh in range(NH):
                    t_m = epool.tile([P, NF], bf16, name="t_m", tag=f"t_m{nh}")
                    s_m = epool.tile([P, NF], bf16, name="s_m", tag=f"s_m{nh}")
                    act_gate(d, "m", AF.Tanh, t_m, nh)
                    act_gate(d, "m", AF.Sigmoid, s_m, nh)
                    sA[(d, "t_m", nh)] = t_m
                    sA[(d, "s_m", nh)] = s_m

            outs = {}

            def dve_cell(d):
                # part = s_i*t_m ; fc = s_m*c ; c_new = fc+part (bf16)
                out_c = opool.tile([P, NH, NF], bf16, name="out_c", tag="out_c")
                outs[(d, "c")] = out_c
                pf = {}
                for nh in range(NH):
                    part = epool.tile([P, NF], bf16, name="part", tag=f"part{nh}")
                    fc = epool.tile([P, NF], bf16, name="fc", tag=f"fc{nh}")
                    nc.vector.tensor_mul(part[:], sA[(d, "s_i", nh)][:],
                                         sA[(d, "t_m", nh)][:])
                    nc.vector.tensor_mul(fc[:], sA[(d, "s_m", nh)][:],
                                         c_tiles[d][:, nh * NF : (nh + 1) * NF])
                    pf[nh] = (part, fc)
                for nh in range(NH):
                    part, fc = pf[nh]
                    nc.vector.tensor_add(out_c[:, nh, :], fc[:], part[:])

            def act_tc(d):
                for nh in range(NH):
                    t_c = epool.tile([P, NF], bf16, name="t_c", tag=f"t_c{nh}")
                    nc.scalar.activation(t_c[:], outs[(d, "c")][:, nh, :], AF.Tanh)
                    sA[(d, "t_c", nh)] = t_c

            def alloc_out_h(d):
                outs[(d, "h")] = opool.tile([P, NH, NF], bf16, name="out_h",
                                            tag="out_h")

            def dve_h(d, nhs=(0, 1)):
                for nh in nhs:
                    nc.vector.tensor_mul(outs[(d, "h")][:, nh, :],
                                         sA[(d, "s_o", nh)][:],
                                         sA[(d, "t_c", nh)][:])

            def dma_out_c(d):
                # c_new is final once dve_cell ran; stream it out while the
                # o-gate matmuls are still running
                nc.sync.dma_start(CNT[d * P : (d + 1) * P, :], outs[(d, "c")][:])

            def dma_out_h(d, nhs=None):
                if nhs is None:
                    nc.sync.dma_start(HT[d * P : (d + 1) * P, :], outs[(d, "h")][:])
                    return
                for nh in nhs:
                    cols = slice(nh * NF, (nh + 1) * NF)
                    nc.sync.dma_start(HT[d * P : (d + 1) * P, cols],
                                      outs[(d, "h")][:, nh, :])

            # ---- d0: chunk-major sweep (i, o, m-main, m-wcorr), gm x-corr
            # deferred past d1 so the PE never waits on the A8l stream.
            # First two chunks run i/o-only so the PE starts as soon as the
            # w_i strip + chunk0 land (the m/lm strips are still streaming);
            # from chunk 2 on, d1's m-gate matmuls interleave in so the PE
            # has >1 chunk of work per arriving chunk and stays saturated.
            alloc_psums(0)
            alloc_psums(1)
            for u in range(2):
                for t in (2 * u, 2 * u + 1):
                    mm_gate(0, "i", t, stop=True)
            for u in range(2):
                for t in (2 * u, 2 * u + 1):
                    mm_m_main(0, t)
                for t in (2 * u, 2 * u + 1):
                    mm_m_wcorr(0, t)
            for u in range(2):
                for t in (2 * u, 2 * u + 1):
                    mm_gate(0, "o", t, stop=True)
            for u in range(2, NCH):
                for g in "io":
                    for t in (2 * u, 2 * u + 1):
                        mm_gate(0, g, t, stop=True)
                for t in (2 * u, 2 * u + 1):
                    mm_m_main(0, t)
                for t in (2 * u, 2 * u + 1):
                    mm_m_wcorr(0, t)
            act_io(0, "i")
            act_io(0, "o")

            # ---- d1: full block minus x-corr
            issue_c(2)
            for t in range(NPAIR):
                mm_m_main(1, t)
                mm_m_wcorr(1, t)
            for t in range(NPAIR):
                mm_gate(1, "i", t, stop=True)
            act_io(1, "i")
            for t in range(NPAIR):
                mm_gate(1, "o", t, stop=True)
            act_io(1, "o")

            # ---- deferred x-corr + phase-B for d0 then d1
            for d01 in (0, 1):
                for t in range(XCORR_PAIRS):
                    mm_m_xcorr(d01, t, stop=(t == XCORR_PAIRS - 1))
                act_m(d01)
                dve_cell(d01)
                act_tc(d01)
                dma_out_c(d01)
                alloc_out_h(d01)
                dve_h(d01)
                dma_out_h(d01)

            # ---- steady state: m first (longest epilogue chain), then i, o
            for d in range(2, DTL):
                if d + 1 < DTL:
                    issue_w(d + 1)
                    issue_c(d + 1)
                alloc_psums(d)
                for t in range(NPAIR):
                    mm_m_main(d, t)
                    if t < XCORR_PAIRS:
                        mm_m_xcorr(d, t)
                    mm_m_wcorr(d, t, stop=True)
                act_m(d)
                for t in range(NPAIR):
                    mm_gate(d, "i", t, stop=True)
                act_io(d, "i")
                dve_cell(d)
                act_tc(d)
                dma_out_c(d)
                alloc_out_h(d)
                if d == DTL - 1:
                    # per-half o sweep: nh0's s_o/h/out stream while nh1's
                    # o matmuls still run (shorter kernel tail)
                    for nh in range(NH):
                        for t in range(NPAIR):
                            mm_gate(d, "o", t, stop=True, nhs=(nh,))
                        act_io(d, "o", nhs=(nh,))
                        dve_h(d, nhs=(nh,))
                        dma_out_h(d, nhs=(nh,))
                else:
                    for t in range(NPAIR):
                        mm_gate(d, "o", t, stop=True)
                    act_io(d, "o")
                    dve_h(d)
                    dma_out_h(d)

    _split_multiwaits(nc)
    return nc


def _get_bass():
    if "nc" not in _CACHE:
        _CACHE["nc"] = _build_bass()
    return _CACHE["nc"]


def _q8(v):
    return np.asarray(np.clip(v, -240.0, 240.0), dtype=_E4M3)


def _prepare_in_maps(x, h, c, Wix, bix, Wmx, bmx, Wox, box, Wih, bih, Wmh, bmh, Woh, boh):
    x = np.asarray(x, dtype=np.float32)
    h = np.asarray(h, dtype=np.float32)
    c = np.asarray(c, dtype=np.float32)

    # W = [Wx ‖ Wh] with gate rows [i, m, o]: [6144, 4096], pre-scaled by SW.
    W_full = np.concatenate(
        [
            np.concatenate([np.asarray(Wix), np.asarray(Wmx), np.asarray(Wox)], axis=0),
            np.concatenate([np.asarray(Wih), np.asarray(Wmh), np.asarray(Woh)], axis=0),
        ],
        axis=1,
    ).astype(np.float32) * SW
    W8 = _q8(W_full)
    # fp8 residual for the m-gate rows only (same scale -> same PSUM).
    W8l_m = _q8(W_full[DH : 2 * DH] - W8[DH : 2 * DH].astype(np.float32))

    # strip[mt] layout: [p, kt, f] with strip[p, kt, f] = W[mt*128+f, kt*128+p]
    WH = np.ascontiguousarray(W8.reshape(MT, P, KT, P).transpose(0, 3, 2, 1))
    WHL = np.ascontiguousarray(W8l_m.reshape(DTL, P, KT, P).transpose(0, 3, 2, 1))
    # WP[d, p, s, kt, f]: s in {i, m, o, lm}
    WP_host = np.ascontiguousarray(
        np.stack([WH[0:DTL], WH[DTL : 2 * DTL], WH[2 * DTL : 3 * DTL], WHL], axis=2)
    )

    # A = [x ‖ h] : [8192, 4096], pre-scaled by SA; fp8 value + fp8 residual.
    A = np.concatenate([x, h], axis=1) * SA
    A8 = _q8(A)
    A8l = _q8(A - A8.astype(np.float32))
    # per-core [p, kt, n]
    A8t = A8.reshape(N_CORES, BLOC, KT, P).transpose(0, 3, 2, 1)
    A8lt = A8l.reshape(N_CORES, BLOC, KT, P).transpose(0, 3, 2, 1)
    AIL_host = np.ascontiguousarray(np.stack([A8t, A8lt], axis=3))

    # c transposed per core, bf16: [core, 2048, 1024]
    CTB_host = np.ascontiguousarray(
        c.reshape(N_CORES, BLOC, DH).transpose(0, 2, 1)
    ).astype(_BF16)

    bias = np.concatenate(
        [
            np.asarray(bix) + np.asarray(bih),
            np.asarray(bmx) + np.asarray(bmh),
            np.asarray(box) + np.asarray(boh),
        ]
    ).astype(np.float32)
    BIAS_host = np.ascontiguousarray(bias.reshape(MT, P).T)

    return [
        {
            "WP": WP_host,
            "AIL": AIL_host[core],
            "CTB": CTB_host[core],
            "BIAS": BIAS_host,
        }
        for core in range(N_CORES)
    ]


def _postprocess(results):
    """results: per-core list of {'HT': [2048,1024] bf16, 'CNT': ...}."""
    h_new = (
        np.stack([np.asarray(results[core]["HT"]) for core in range(N_CORES)])
        .astype(np.float32)
        .transpose(0, 2, 1)
        .reshape(B, DH)
    )
    c_new = (
        np.stack([np.asarray(results[core]["CNT"]) for core in range(N_CORES)])
        .astype(np.float32)
        .transpose(0, 2, 1)
        .reshape(B, DH)
    )
    return (h_new, c_new)


def kernel(x, h, c, Wix, bix, Wmx, bmx, Wox, box, Wih, bih, Wmh, bmh, Woh, boh):
    global LAST_RESULT
    from concourse.bass_utils import run_bass_kernel_spmd

    in_maps = _prepare_in_maps(
        x, h, c, Wix, bix, Wmx, bmx, Wox, box, Wih, bih, Wmh, bmh, Woh, boh
    )
    nc = _get_bass()
    try:
        res = run_bass_kernel_spmd(nc, in_maps, core_ids=list(range(N_CORES)))
    except ModuleNotFoundError:
        # BASS_TRACE under axon needs antenv.axon_hooks, which some
        # containers lack; fall back to an untraced run.
        os.environ["BASS_NEVER_TRACE"] = "1"
        res = run_bass_kernel_spmd(nc, in_maps, core_ids=list(range(N_CORES)))
    LAST_RESULT = res
    return _postprocess(res.results)


# revision 37
# speedup vs baseline: 1.0095x; 1.0095x over previous
"""LSTM cell kernel for Trainium2, SPMD over 8 NeuronCores — fp8 DoubleRow.

Problem: nn_LstmCell — B=8192, D_IN=D_H=2048.
    g = x @ Wx.T + bx + h @ Wh.T + bh          # [B, 3H]
    gi, gm, go = split(g, 3)
    c_new = sigmoid(gm)*c + sigmoid(gi)*tanh(gm)
    h_new = sigmoid(go)*tanh(c_new)

Strategy:
  - Data-parallel over batch: each core owns 1024 rows of x/h/c.
  - Single fused GEMM A = [x ‖ h] (K=4096) against W = [Wx ‖ Wh] ([6144, 4096]),
    computed transposed (gates on PSUM partitions, batch on free dim).
  - fp8(e4m3) matmuls in MatmulPerfMode.DoubleRow: one instruction contracts a
    PAIR of 128-deep k-planes at 0.5 cycles per output row — 4x the bf16 rate.
  - Mixed precision to stay inside the error budget: the memory gate (gm,
    whose pre-activation feeds both sigmoid and tanh) gets a first-order
    quantization-error correction; gi/go run raw fp8.
      A = A8 + A8l, W = W8 + W8l (fp8 residual decomposition, same scale so
      everything accumulates in one PSUM):
        gm  ≈ W8ᵀA8 + W8lᵀA8 (all 16 k-pairs) + W8ᵀA8l (first XCORR_PAIRS)
        gi/go ≈ W8ᵀA8                        (1 DR matmul per pair)
    Measured on HW (bit-exact vs the numpy pipeline sim): h_new rel err
    1.890e-2 at XCORR_PAIRS=8 (budget 2e-2).
  - Schedule: m -> i -> o per d-tile with eager per-gate PSUM drains; m-gate
    PSUM banks double-buffered by d-parity (8 banks exactly); d0 consumes A8
    chunks as they stream, d0/d1 defer the x-corr until the A8l planes land;
    weight/c/out DMAs sequenced so the serialized DMA engines never starve
    the PE after startup.
  - Per-gate bias and the 2^-18 dequant scale fold into the ScalarE
    activation (func(in*scale + bias) reads PSUM directly).
  - c/outputs and the elementwise epilogue stay in f32 (DVE/DMA have
    slack); the saved error budget pays for one fewer x-corr pair.

Host-side: layout transforms + fp8/bf16 casts (not counted in HW exec time).
"""

import os

import numpy as np
import ml_dtypes

N_CORES = 8
B = 8192
DH = 2048            # latent dim (= D_IN = D_H)
H3 = 3 * DH          # 6144 gate rows
K = 2 * DH           # 4096 contraction dim
BLOC = B // N_CORES  # 1024 batch rows per core
P = 128
KT = K // P          # 32 k-planes
NPAIR = KT // 2      # 16 DoubleRow k-plane pairs
MT = H3 // P         # 48 gate-row tiles
DTL = DH // P        # 16 d-tiles per gate
NF = 512             # matmul free dim (one PSUM bank of fp32)
NH = BLOC // NF      # 2 batch halves

SA = 32.0            # fp8 scale for A (|A|max ~5.1 -> 163 < 240)
SW = 8192.0          # fp8 scale for W (|W|max 0.0221 -> 181 < 240)
SC = 1.0 / (SA * SW)  # dequant scale folded into ACT

# k-pairs (of 16) that get the gm x-residual correction. With the f32
# epilogue: 8 -> h_rel 1.851e-2, 7 -> 1.896e-2, 6 -> 1.939e-2 (exact-pipeline
# sim, reproduces HW bit-for-bit), budget 2e-2.
XCORR_PAIRS = 7

_BF16 = ml_dtypes.bfloat16
_E4M3 = ml_dtypes.float8_e4m3

_CACHE = {}
LAST_RESULT = None  # BassKernelResults from the most recent run (for test.py)


def _split_multiwaits(nc):
    """This container's walrus build rejects >1 sync-wait on an engine
    instruction ("Too many sync wait commands"). Split extra waits into
    standalone EventSemaphore instructions on the same engine immediately
    before the instruction (same stall semantics: engines are in-order)."""
    import concourse.mybir as mybir

    f = nc.m.functions[0]
    for blk in f.blocks:
        new_insts = []
        for inst in blk.instructions:
            si = getattr(inst, "sync_info", None)
            ow = list(si.on_wait) if (si is not None and si.on_wait) else []
            if len(ow) > 1:
                for w in ow[:-1]:
                    new_insts.append(
                        mybir.InstEventSemaphore(
                            name=nc.get_next_instruction_name(),
                            engine=inst.engine,
                            ins=[],
                            outs=[],
                            sync_info=mybir.SyncInfo(on_wait=[w], on_update=[]),
                        )
                    )
                inst.sync_info = mybir.SyncInfo(
                    on_wait=[ow[-1]], on_update=list(si.on_update)
                )
            new_insts.append(inst)
        blk.instructions[:] = new_insts


def _build_bass():
    import concourse.bass as bass
    import concourse.mybir as mybir
    import concourse.tile as tile

    f32 = mybir.dt.float32
    bf16 = mybir.dt.bfloat16
    fp8 = mybir.dt.float8e4
    AF = mybir.ActivationFunctionType
    DR = mybir.MatmulPerfMode.DoubleRow

    nc = bass.Bass("TRN2", name="lstm_cell_fp8")

    # WP[d]: per-partition-contiguous pack of the 4 weight strips this d-tile
    # needs: [w8_i, w8_m, w8_o, w8l_m]. WP[d, p, s, kt, f] = strip_s[kt*128+p,
    # d-tile row f].
    WP = nc.dram_tensor("WP", [DTL, P, 4, KT, P], fp8, kind="ExternalInput")
    # AIL[p, kt, 0, n] = A8, AIL[p, kt, 1, n] = A8 residual.
    AIL = nc.dram_tensor("AIL", [P, KT, 2, BLOC], fp8, kind="ExternalInput")
    CTB = nc.dram_tensor("CTB", [DH, BLOC], f32, kind="ExternalInput")
    BIAS = nc.dram_tensor("BIAS", [P, MT], f32, kind="ExternalInput")
    HT = nc.dram_tensor("HT", [DH, BLOC], f32, kind="ExternalOutput")
    CNT = nc.dram_tensor("CNT", [DH, BLOC], f32, kind="ExternalOutput")

    NCH = 8  # A-chunk DMAs per plane (2 k-pairs each)

    with tile.TileContext(nc) as tc:
        with (
            tc.tile_pool(name="const", bufs=1) as const_pool,
            tc.tile_pool(name="wpool", bufs=2) as wpool,
            tc.tile_pool(name="cpool", bufs=2) as cpool,
            tc.tile_pool(name="epool", bufs=2) as epool,
            tc.tile_pool(name="opool", bufs=2) as opool,
            tc.tile_pool(name="psum", bufs=1, space="PSUM") as psum_pool,
        ):
            # Activations resident in SBUF. A8 planes stream first (all
            # matmuls need them); A8l residual planes later (only the gm
            # x-correction reads them, and d0/d1 defer that).
            a_sb = const_pool.tile([P, KT, 2, BLOC], fp8, name="a_sb")

            # d0 weights as 4 separate strips (i first) so the first
            # matmuls start ~1.5us in instead of behind a 5.8us pack.
            strips0 = {}
            for si, sname in [(0, "i"), (1, "m"), (3, "lm"), (2, "o")]:
                st = const_pool.tile([P, KT, P], fp8, name=f"w0_{sname}")
                nc.sync.dma_start(st[:], WP[0][:, si])
                strips0[sname] = st
            bias_sb = const_pool.tile([P, MT], f32, name="bias_sb")
            nc.sync.dma_start(bias_sb[:], BIAS[:])

            w_tiles = {}
            c_tiles = {}

            def issue_w(d, strips=None):
                # strips: subset of slots to DMA on the in-order Pool queue
                # (sequencing them between A-chunk streams controls exactly
                # when they reach the serialized DMA engines); None = one
                # pack DMA on the SP queue.
                wp = w_tiles.get(d)
                if wp is None:
                    wp = wpool.tile([P, 4, KT, P], fp8, name="wp", tag="wp")
                    w_tiles[d] = wp
                if strips is None:
                    nc.sync.dma_start(wp[:], WP[d])
                else:
                    for s in strips:
                        nc.gpsimd.dma_start(wp[:, s], WP[d][:, s])

            def issue_c(d, pool_queue=False):
                c_t = cpool.tile([P, BLOC], f32, name="c_t", tag="c_t")
                eng = nc.gpsimd if pool_queue else nc.sync
                eng.dma_start(c_t[:], CTB[d * P : (d + 1) * P, :])
                c_tiles[d] = c_t

            # Pool queue is in-order; sequence the startup byte stream so PE
            # work becomes available as early as possible: A8 chunks with d1's
            # m/lm strips early (d1-m matmuls interleave into the d0 sweep),
            # then the rest of d1, the A8l residual planes, d2, and the c's.
            def a_chunk(u, plane):
                nc.gpsimd.dma_start(
                    a_sb[:, 4 * u : 4 * u + 4, plane, :],
                    AIL[:, 4 * u : 4 * u + 4, plane, :],
                )

            for u in range(NCH):
                a_chunk(u, 0)
            issue_w(1, strips=(1, 3, 0, 2))
            for u in range((2 * XCORR_PAIRS + 3) // 4):  # chunks x-corr reads
                a_chunk(u, 1)
            issue_w(2, strips=(1, 3, 0, 2))
            issue_c(0, pool_queue=True)
            issue_c(1, pool_queue=True)

            def w_slice(d, s, t):
                if d == 0:
                    st = strips0[{0: "i", 1: "m", 2: "o", 3: "lm"}[s]]
                    return st[:, 2 * t : 2 * t + 2, :]
                return w_tiles[d][:, s, 2 * t : 2 * t + 2, :]

            def a_pair(t, plane, nh):
                return a_sb[:, 2 * t : 2 * t + 2, plane,
                            nh * NF : (nh + 1) * NF]

            def mm(psum, lhsT, rhs, start, stop):
                nc.tensor.matmul(psum, lhsT, rhs, start=start, stop=stop,
                                 perf_mode=DR)

            psums = {}

            def alloc_psums(d):
                # i/o banks single-buffered (recycled after the eager
                # s_i/s_o ACT read); m banks double-buffered by d-parity so
                # consecutive d-tiles' m accumulations never serialize.
                for g in "io":
                    for nh in range(NH):
                        psums[(d, g, nh)] = psum_pool.tile(
                            [P, NF], f32, name=f"ps_{g}{nh}", tag=f"ps_{g}{nh}"
                        )
                for nh in range(NH):
                    psums[(d, "m", nh)] = psum_pool.tile(
                        [P, NF], f32, name=f"ps_m{nh}", tag=f"ps_m{nh}_{d % 2}"
                    )

            def mm_gate(d, g, t, stop=False, nhs=(0, 1)):
                # plain gate: one DR per (t, nh) on the A8 plane
                gi = {"i": 0, "m": 1, "o": 2}[g]
                for nh in nhs:
                    mm(psums[(d, g, nh)][:], w_slice(d, gi, t),
                       a_pair(t, 0, nh), t == 0, stop and t == NPAIR - 1)

            def mm_m_main(d, t, start=None):
                for nh in range(NH):
                    mm(psums[(d, "m", nh)][:], w_slice(d, 1, t),
                       a_pair(t, 0, nh), t == 0 if start is None else start,
                       False)

            def mm_m_xcorr(d, t, stop=False):
                for nh in range(NH):
                    mm(psums[(d, "m", nh)][:], w_slice(d, 1, t),
                       a_pair(t, 1, nh), False, stop)

            def mm_m_wcorr(d, t, stop=False):
                for nh in range(NH):
                    mm(psums[(d, "m", nh)][:], w_slice(d, 3, t),
                       a_pair(t, 0, nh), False, stop and t == NPAIR - 1)

            def act_gate(d, g, func, dst, nh):
                col = {"i": d, "m": DTL + d, "o": 2 * DTL + d}[g]
                nc.scalar.activation(dst[:], psums[(d, g, nh)][:], func,
                                     bias=bias_sb[:, col : col + 1], scale=SC)

            sA = {}  # (d, name, nh) -> bf16 tile for s_i/t_m/s_m/s_o

            def act_io(d, g, nhs=(0, 1)):
                # eager phase-A drain: frees the i/o bank for d+1
                for nh in nhs:
                    tl = epool.tile([P, NF], f32, name=f"s_{g}", tag=f"s_{g}{nh}")
                    act_gate(d, g, AF.Sigmoid, tl, nh)
                    sA[(d, f"s_{g}", nh)] = tl

            def act_m(d):
                for nh in range(NH):
                    t_m = epool.tile([P, NF], f32, name="t_m", tag=f"t_m{nh}")
                    s_m = epool.tile([P, NF], f32, name="s_m", tag=f"s_m{nh}")
                    act_gate(d, "m", AF.Tanh, t_m, nh)
                    act_gate(d, "m", AF.Sigmoid, s_m, nh)
                    sA[(d, "t_m", nh)] = t_m
                    sA[(d, "s_m", nh)] = s_m

            outs = {}

            def dve_fc(d):
                # fc = s_m*c is ready at the m-gate ACTs — compute it while
                # the i-gate matmuls run, off the s_i->part->add chain
                for nh in range(NH):
                    fc = epool.tile([P, NF], f32, name="fc", tag=f"fc{nh}")
                    nc.vector.tensor_mul(fc[:], sA[(d, "s_m", nh)][:],
                                         c_tiles[d][:, nh * NF : (nh + 1) * NF])
                    sA[(d, "fc", nh)] = fc

            def dve_cell(d):
                # part = s_i*t_m ; c_new = fc+part
                out_c = opool.tile([P, NH, NF], f32, name="out_c", tag="out_c")
                outs[(d, "c")] = out_c
                for nh in range(NH):
                    part = epool.tile([P, NF], f32, name="part", tag=f"part{nh}")
                    nc.vector.tensor_mul(part[:], sA[(d, "s_i", nh)][:],
                                         sA[(d, "t_m", nh)][:])
                    nc.vector.tensor_add(out_c[:, nh, :], sA[(d, "fc", nh)][:],
                                         part[:])

            def act_tc(d):
                for nh in range(NH):
                    t_c = epool.tile([P, NF], f32, name="t_c", tag=f"t_c{nh}")
                    nc.scalar.activation(t_c[:], outs[(d, "c")][:, nh, :], AF.Tanh)
                    sA[(d, "t_c", nh)] = t_c

            def alloc_out_h(d):
                outs[(d, "h")] = opool.tile([P, NH, NF], f32, name="out_h",
                                            tag="out_h")

            def dve_h(d, nhs=(0, 1)):
                for nh in nhs:
                    nc.vector.tensor_mul(outs[(d, "h")][:, nh, :],
                                         sA[(d, "s_o", nh)][:],
                                         sA[(d, "t_c", nh)][:])

            def dma_out_c(d):
                # c_new is final once dve_cell ran; stream it out while the
                # o-gate matmuls are still running
                nc.sync.dma_start(CNT[d * P : (d + 1) * P, :], outs[(d, "c")][:])

            def dma_out_h(d, nhs=None):
                if nhs is None:
                    nc.sync.dma_start(HT[d * P : (d + 1) * P, :], outs[(d, "h")][:])
                    return
                for nh in nhs:
                    cols = slice(nh * NF, (nh + 1) * NF)
                    nc.sync.dma_start(HT[d * P : (d + 1) * P, cols],
                                      outs[(d, "h")][:, nh, :])

            # ---- d0: chunk-major sweep (i, o, m-main, m-wcorr), gm x-corr
            # deferred past d1 so the PE never waits on the A8l stream.
            # First two chunks run i/o-only so the PE starts as soon as the
            # w_i strip + chunk0 land (the m/lm strips are still streaming);
            # from chunk 2 on, d1's m-gate matmuls interleave in so the PE
            # has >1 chunk of work per arriving chunk and stays saturated.
            alloc_psums(0)
            alloc_psums(1)
            for u in range(2):
                for t in (2 * u, 2 * u + 1):
                    mm_gate(0, "i", t, stop=True)
            for u in range(2):
                for t in (2 * u, 2 * u + 1):
                    mm_m_main(0, t)
                for t in (2 * u, 2 * u + 1):
                    mm_m_wcorr(0, t)
            for u in range(2):
                for t in (2 * u, 2 * u + 1):
                    mm_gate(0, "o", t, stop=True)
            for u in range(2, NCH):
                for g in "io":
                    for t in (2 * u, 2 * u + 1):
                        mm_gate(0, g, t, stop=True)
                for t in (2 * u, 2 * u + 1):
                    mm_m_main(0, t)
                for t in (2 * u, 2 * u + 1):
                    mm_m_wcorr(0, t)
            act_io(0, "i")
            act_io(0, "o")

            # ---- d1: full block minus x-corr
            issue_c(2)
            for t in range(NPAIR):
                mm_m_main(1, t)
                mm_m_wcorr(1, t)
            for t in range(NPAIR):
                mm_gate(1, "i", t, stop=True)
            act_io(1, "i")
            for t in range(NPAIR):
                mm_gate(1, "o", t, stop=True)
            act_io(1, "o")

            # ---- deferred x-corr + phase-B for d0 then d1
            for d01 in (0, 1):
                for t in range(XCORR_PAIRS):
                    mm_m_xcorr(d01, t, stop=(t == XCORR_PAIRS - 1))
                act_m(d01)
                dve_fc(d01)
                dve_cell(d01)
                act_tc(d01)
                dma_out_c(d01)
                alloc_out_h(d01)
                dve_h(d01)
                dma_out_h(d01)

            # ---- steady state: m first (longest epilogue chain), then i, o
            for d in range(2, DTL):
                if d + 1 < DTL:
                    issue_w(d + 1)
                    issue_c(d + 1)
                alloc_psums(d)
                for t in range(NPAIR):
                    mm_m_main(d, t)
                    if t < XCORR_PAIRS:
                        mm_m_xcorr(d, t)
                    mm_m_wcorr(d, t, stop=True)
                act_m(d)
                dve_fc(d)
                for t in range(NPAIR):
                    mm_gate(d, "i", t, stop=True)
                act_io(d, "i")
                dve_cell(d)
                act_tc(d)
                dma_out_c(d)
                alloc_out_h(d)
                if d == DTL - 1:
                    # per-half o sweep: nh0's s_o/h/out stream while nh1's
                    # o matmuls still run; nh1 additionally splits into two
                    # 256-column PSUM slices (both sweeps issued before the
                    # ACTs to avoid a whole-tile WAR stall) so slice A's
                    # s_o/h/DMA chain overlaps slice B's matmuls.
                    for t in range(NPAIR):
                        mm_gate(d, "o", t, stop=True, nhs=(0,))
                    act_io(d, "o", nhs=(0,))
                    dve_h(d, nhs=(0,))
                    dma_out_h(d, nhs=(0,))
                    ps_o1 = psums[(d, "o", 1)]
                    col_b = 2 * DTL + d
                    for half in range(2):
                        cs = slice(half * (NF // 2), (half + 1) * (NF // 2))
                        for t in range(NPAIR):
                            mm(ps_o1[:, cs], w_slice(d, 2, t),
                               a_sb[:, 2 * t : 2 * t + 2, 0,
                                    NF + half * (NF // 2) :
                                    NF + (half + 1) * (NF // 2)],
                               t == 0, t == NPAIR - 1)
                    for half in range(2):
                        cs = slice(half * (NF // 2), (half + 1) * (NF // 2))
                        s_oh = epool.tile([P, NF // 2], f32, name="s_o",
                                          tag=f"s_oh{half}")
                        nc.scalar.activation(s_oh[:], ps_o1[:, cs], AF.Sigmoid,
                                             bias=bias_sb[:, col_b : col_b + 1],
                                             scale=SC)
                        t_ch = sA[(d, "t_c", 1)]
                        nc.vector.tensor_mul(outs[(d, "h")][:, 1, cs],
                                             s_oh[:], t_ch[:, cs])
                        nc.sync.dma_start(
                            HT[d * P : (d + 1) * P,
                               NF + half * (NF // 2) :
                               NF + (half + 1) * (NF // 2)],
                            outs[(d, "h")][:, 1, cs])
                else:
                    for t in range(NPAIR):
                        mm_gate(d, "o", t, stop=True)
                    act_io(d, "o")
                    dve_h(d)
                    dma_out_h(d)

    _split_multiwaits(nc)
    return nc


def _get_bass():
    if "nc" not in _CACHE:
        _CACHE["nc"] = _build_bass()
    return _CACHE["nc"]


def _q8(v):
    return np.asarray(np.clip(v, -240.0, 240.0), dtype=_E4M3)


def _prepare_in_maps(x, h, c, Wix, bix, Wmx, bmx, Wox, box, Wih, bih, Wmh, bmh, Woh, boh):
    x = np.asarray(x, dtype=np.float32)
    h = np.asarray(h, dtype=np.float32)
    c = np.asarray(c, dtype=np.float32)

    # W = [Wx ‖ Wh] with gate rows [i, m, o]: [6144, 4096], pre-scaled by SW.
    W_full = np.concatenate(
        [
            np.concatenate([np.asarray(Wix), np.asarray(Wmx), np.asarray(Wox)], axis=0),
            np.concatenate([np.asarray(Wih), np.asarray(Wmh), np.asarray(Woh)], axis=0),
        ],
        axis=1,
    ).astype(np.float32) * SW
    W8 = _q8(W_full)
    # fp8 residual for the m-gate rows only (same scale -> same PSUM).
    W8l_m = _q8(W_full[DH : 2 * DH] - W8[DH : 2 * DH].astype(np.float32))

    # strip[mt] layout: [p, kt, f] with strip[p, kt, f] = W[mt*128+f, kt*128+p]
    WH = np.ascontiguousarray(W8.reshape(MT, P, KT, P).transpose(0, 3, 2, 1))
    WHL = np.ascontiguousarray(W8l_m.reshape(DTL, P, KT, P).transpose(0, 3, 2, 1))
    # WP[d, p, s, kt, f]: s in {i, m, o, lm}
    WP_host = np.ascontiguousarray(
        np.stack([WH[0:DTL], WH[DTL : 2 * DTL], WH[2 * DTL : 3 * DTL], WHL], axis=2)
    )

    # A = [x ‖ h] : [8192, 4096], pre-scaled by SA; fp8 value + fp8 residual.
    A = np.concatenate([x, h], axis=1) * SA
    A8 = _q8(A)
    A8l = _q8(A - A8.astype(np.float32))
    # per-core [p, kt, n]
    A8t = A8.reshape(N_CORES, BLOC, KT, P).transpose(0, 3, 2, 1)
    A8lt = A8l.reshape(N_CORES, BLOC, KT, P).transpose(0, 3, 2, 1)
    AIL_host = np.ascontiguousarray(np.stack([A8t, A8lt], axis=3))

    # c transposed per core, bf16: [core, 2048, 1024]
    CTB_host = np.ascontiguousarray(
        c.reshape(N_CORES, BLOC, DH).transpose(0, 2, 1)
    )

    bias = np.concatenate(
        [
            np.asarray(bix) + np.asarray(bih),
            np.asarray(bmx) + np.asarray(bmh),
            np.asarray(box) + np.asarray(boh),
        ]
    ).astype(np.float32)
    BIAS_host = np.ascontiguousarray(bias.reshape(MT, P).T)

    return [
        {
            "WP": WP_host,
            "AIL": AIL_host[core],
            "CTB": CTB_host[core],
            "BIAS": BIAS_host,
        }
        for core in range(N_CORES)
    ]


def _postprocess(results):
    """results: per-core list of {'HT': [2048,1024] bf16, 'CNT': ...}."""
    h_new = (
        np.stack([np.asarray(results[core]["HT"]) for core in range(N_CORES)])
        .astype(np.float32)
        .transpose(0, 2, 1)
        .reshape(B, DH)
    )
    c_new = (
        np.stack([np.asarray(results[core]["CNT"]) for core in range(N_CORES)])
        .astype(np.float32)
        .transpose(0, 2, 1)
        .reshape(B, DH)
    )
    return (h_new, c_new)


def kernel(x, h, c, Wix, bix, Wmx, bmx, Wox, box, Wih, bih, Wmh, bmh, Woh, boh):
    global LAST_RESULT
    from concourse.bass_utils import run_bass_kernel_spmd

    in_maps = _prepare_in_maps(
        x, h, c, Wix, bix, Wmx, bmx, Wox, box, Wih, bih, Wmh, bmh, Woh, boh
    )
    nc = _get_bass()
    try:
        res = run_bass_kernel_spmd(nc, in_maps, core_ids=list(range(N_CORES)))
    except ModuleNotFoundError:
        # BASS_TRACE under axon needs antenv.axon_hooks, which some
        # containers lack; fall back to an untraced run.
        os.environ["BASS_NEVER_TRACE"] = "1"
        res = run_bass_kernel_spmd(nc, in_maps, core_ids=list(range(N_CORES)))
    LAST_RESULT = res
    return _postprocess(res.results)
